# revision 72
# baseline (speedup 1.0000x reference)
"""Trainium2 Bass kernel for nn_CMAModel (control-fused memory attention).

Math (reference):
  q  = x @ Wq.T + ctrl @ Wc.T                  [B,T,C]
  kv = [x; fwd_mem; rev_mem]                   [B,S,C], S = T+M+R = 5440
  k  = kv @ Wk.T ; v = kv @ Wv.T
  per head h (D=128): scores = q_h k_h^T / sqrt(D), causal mask on the
  local T block only; w = softmax(scores); out_h = w_loc v_loc + gate_h *
  (w_mem v_mem); gate = sigmoid(q @ Wg.T + bg); y = concat(out_h) @ Wo.T

Sharding (8 cores, SPMD — one program, per-core behavior via input data):
  core = b*4 + g  (b = batch, g = group 0..3).  24 units of (b, head,
  T-half).  Each core runs 3 "slots": slots 0,1 = both halves of a
  "pair" head, slot 2 = one half of a "single" head (shared with the
  neighbor core).

Design (v6, pipelined, ~228.5 us; v3 was ~248 us):
  - Tail-slot finalize reads the AV accumulators directly from psum
    (no Lsb copies) and the boundary gate matmuls use the freed m-acc
    banks, keeping the scores rotation clean.
Earlier design notes (v4/v5):
  - All attention-path data fp16.  Tiny control projections on host.
  - kvT DRAM layout is chunk-major contiguous slabs (6KB/partition
    runs) split across the sync+scalar HWDGE queues -> ~2x DMA bw.
  - LOCAL chunks first (q + gate ready early), then MEM chunks with
    slot-0's memory-attention tiles interleaved right into the KV loop:
    exp (ACT) and Rt adds (DVE) start ~45us earlier, KV->attention
    phase bubble + HAM re-throttle eliminated.
  - ONE psum pool for the whole kernel (8 banks exactly):
      tag pk0 [P,2,512] x2 bufs (4 banks): warmup, K pairs, q, ALL
        scores, finalize projections
      tag pg / av1 (2 banks): gate rows, then AV accumulators ch0/ch1
      tag pv0/pv1 (2 banks): V psums during KV, AV-local after
  - Causal masking via -30000 bias accumulated INTO scores psum by an
    identity matmul against host-built shifted step tables.
  - Softmax denominators NOT applied on device: per-slot fp16 Rt
    partial sums ship to host; unshard scales column-wise.
  - Slot finalize split into small steps interleaved into the next
    slot's tiles; finalize copies on DVE (ACT stays exp-only).
"""

import numpy as np

B, T, C, H, M, R = 2, 2048, 768, 6, 3072, 320
D = C // H          # 128
S = T + M + R       # 5440
P = 128
NT = (S + P - 1) // P          # 43 s-tiles (last has 64 rows)
NLOC = T // P                  # 16 local s-tiles
NCT = C // P                   # 6 feature tiles
THALF = T // 2                 # 1024
NCH = THALF // 512             # 2 chunks of 512 per half
DSCALE = float(D) ** -0.5
import os as _os
SKIP_MASKED = int(_os.environ.get("SKIP_MASKED", "1"))
EARLY_FIN = int(_os.environ.get("EARLY_FIN", "0"))
NSTASH = int(_os.environ.get("NSTASH", "0"))
STASH_JS = list(range(NT - NSTASH, NT))   # last mem tiles of slot 1

# chunk schedule: local first (q/gate), then memory
CHUNKS = [(0, 1024), (1024, 1024), (2048, 1024), (3072, 1024),
          (4096, 1024), (5120, 320)]
N_LOC_CHUNKS = 2


def _chunk_layout():
    """flat kvT column offsets: chunk-major, sub(512)-major, ct-major."""
    bases = []
    off = 0
    for _, w in CHUNKS:
        subs = []
        o2 = 0
        while o2 < w:
            sw = min(512, w - o2)
            subs.append((off, o2, sw))
            off += NCT * sw
            o2 += sw
        bases.append(subs)
    return bases, off


CHUNK_SUBS, KVT_COLS = _chunk_layout()

# per-batch slot maps: (pair_head, single_head, single_half) per group
GROUP_MAP = [(0, 1, 0), (2, 1, 1), (3, 4, 0), (5, 4, 1)]


def slot_units(g):
    hp, hs, hsh = GROUP_MAP[g]
    return [(hp, 0), (hp, 1), (hs, hsh)]


def build_nc(debug=False):
    import concourse.mybir as mybir
    import concourse.tile as tile
    from concourse import bacc

    f32 = mybir.dt.float32
    f16 = mybir.dt.float16
    AF = mybir.ActivationFunctionType
    OP = mybir.AluOpType

    nc = bacc.Bacc("TRN2", target_bir_lowering=False, debug=False,
                   num_devices=8)

    def mm(psum, lhsT, rhs, start=True, stop=True):
        nc.tensor.matmul(psum, lhsT, rhs, start=start, stop=stop)

    dram = {}
    for name, shape, dt_ in [
        ("kvT", [P, KVT_COLS], f16),       # chunk-major slabs
        ("xq2", [P, NCT * THALF], f16),    # slot-2 q source columns
        ("wk0", [P, NCT * P], f16),        # pair-head Wk, [p, ct, m]
        ("wk1", [P, NCT * P], f16),        # single-head Wk
        ("wv2", [P, NCT * 2 * P], f16),    # [p, ct, 2 heads * 128]
        ("wq3", [P, NCT * 2 * P], f16),    # [p, ct, (own|s2)*128]
        ("wfg", [P, NCT * 4 * 3], f16),    # [p, ct, chunk, row] gate w
        ("wot", [P, 3 * C], f16),          # [d, slot*C + c]
        ("ident", [P, P], f16),            # identity for PE bias matmul
        ("g01", [P, 2 * THALF], f16),      # causal bias table slots 0/1
        ("g2", [P, 3 * THALF], f16),       # per-core slot-2 bias table
        ("qbs", [P, 3], f32),              # per-slot q bias col
        ("gb3", [3, 1], f32),              # gate bias rows (3 used)
    ]:
        dram[name] = nc.dram_tensor(name, shape, dt_, kind="ExternalInput")
    yp = nc.dram_tensor("yp", [3 * C, THALF], f16, kind="ExternalOutput")
    rts = nc.dram_tensor("rts", [3 * P, THALF], f16, kind="ExternalOutput")
    dbg = {}
    if debug:
        for name, shape in [("d_q", [P, 3 * THALF]),
                            ("d_gate", [3, THALF]),
                            ("d_kh0", [P, 1024]), ("d_vh", [P, 512]),
                            ("d_att", [P, 3 * THALF])]:
            dbg[name] = nc.dram_tensor(name, shape, f32,
                                       kind="ExternalOutput")

    from contextlib import ExitStack

    with tile.TileContext(nc) as tc, ExitStack() as _ctx:
        consts = _ctx.enter_context(tc.tile_pool(name="consts", bufs=1))
        # ---- constants into SBUF ----
        # critical set for chunk-0 (2.8MB) split across sync+scalar+gpsimd
        # rings; everything else deferred so it doesn't steal HBM bw.
        kvsrc = dram["kvT"][:, :]
        b0 = CHUNK_SUBS[0][0][0]
        b1 = CHUNK_SUBS[0][1][0]
        wk0 = consts.tile([P, NCT, P], f16)
        nc.gpsimd.dma_start(out=wk0[:], in_=dram["wk0"][:, :].rearrange(
            "p (a m) -> p a m", a=NCT))
        wk1 = consts.tile([P, NCT, P], f16)
        nc.gpsimd.dma_start(out=wk1[:], in_=dram["wk1"][:, :].rearrange(
            "p (a m) -> p a m", a=NCT))
        # first kv slab ct-split across both HWDGE rings (each ring only
        # sustains ~100GB/s, so halve the per-ring critical bytes)
        kvt0a = consts.tile([P, NCT, 512], f16)
        nc.sync.dma_start(
            out=kvt0a[:, 0:3, :],
            in_=kvsrc[:, b0:b0 + 3 * 512].rearrange(
                "p (a s) -> p a s", a=3))
        nc.scalar.dma_start(
            out=kvt0a[:, 3:6, :],
            in_=kvsrc[:, b0 + 3 * 512:b0 + 6 * 512].rearrange(
                "p (a s) -> p a s", a=3))
        kvt0b = consts.tile([P, NCT, 512], f16)
        nc.sync.dma_start(
            out=kvt0b[:, 0:3, :],
            in_=kvsrc[:, b1:b1 + 3 * 512].rearrange(
                "p (a s) -> p a s", a=3))
        nc.scalar.dma_start(
            out=kvt0b[:, 3:6, :],
            in_=kvsrc[:, b1 + 3 * 512:b1 + 6 * 512].rearrange(
                "p (a s) -> p a s", a=3))
        qbs = consts.tile([P, 3], f32)
        nc.gpsimd.dma_start(out=qbs[:], in_=dram["qbs"][:, :])
        gb3 = consts.tile([3, 1], f32)
        nc.gpsimd.dma_start(out=gb3[:], in_=dram["gb3"][:, :])
        wv2 = consts.tile([P, NCT, 2 * P], f16)
        nc.gpsimd.dma_start(out=wv2[:], in_=dram["wv2"][:, :].rearrange(
            "p (a m) -> p a m", a=NCT))
        wq3 = consts.tile([P, NCT, 2 * P], f16)
        nc.sync.dma_start(out=wq3[:], in_=dram["wq3"][:, :].rearrange(
            "p (a m) -> p a m", a=NCT))
        wfg = consts.tile([P, NCT, 4, 3], f16)
        nc.scalar.dma_start(out=wfg[:], in_=dram["wfg"][:, :].rearrange(
            "p (a c r) -> p a c r", a=NCT, c=4))
        ones_c16 = consts.tile([P, 1], f16)
        nc.vector.memset(ones_c16[:], 1.0)
        ones_r16 = consts.tile([1, P], f16)
        nc.vector.memset(ones_r16[:], 1.0)
        # remaining consts stream on gpsimd (needed from ~85us on)
        wot = consts.tile([P, 3 * C], f16)
        ident = consts.tile([P, P], f16)
        g01 = consts.tile([P, 2 * THALF], f16)
        g2 = consts.tile([P, 3 * THALF], f16)
        xq2 = consts.tile([P, NCT, THALF], f16)
        gscr = consts.tile([1, 8], f16)
        lateg = consts.tile([1, 8], f16)
        # slot-1 deferred-AV stash: exps computed during the KV phase
        # (ACT slack there), AVs drained in the exp-bound tail phase
        estash = consts.tile([P, max(NSTASH, 1), THALF], f16)

        # ---- outputs of the kv+q phase ----
        kh0 = consts.tile([P, S], f16)
        kh1 = consts.tile([P, S], f16)
        vh = consts.tile([P, NT, 2 * P], f16)
        qsb = consts.tile([P, 3, THALF], f16)
        gacc = consts.tile([3, THALF], f32)   # gate logits rows 0..2
        gate = consts.tile([3, THALF], f16)
        gate1 = consts.tile([1, 3, THALF], f16)  # partition-0 re-layout

        # ---- pools (single scope; one psum pool, 8 banks exactly) ----
        kvp = _ctx.enter_context(tc.tile_pool(name="kvp", bufs=2))
        kvps = _ctx.enter_context(
            tc.tile_pool(name="kvps", bufs=1, space="PSUM"))
        att_pool = _ctx.enter_context(tc.tile_pool(name="att", bufs=2))
        ep = _ctx.enter_context(tc.tile_pool(name="ep", bufs=8))
        vec = _ctx.enter_context(tc.tile_pool(name="vec", bufs=3))
        cmb = _ctx.enter_context(tc.tile_pool(name="cmb", bufs=2))
        ysb = _ctx.enter_context(tc.tile_pool(name="ysb", bufs=3))

        # PE warmup while first DMAs land (double-buffered: no WAW stall)
        wu = kvp.tile([P, 512], f16, tag="wu", bufs=1)
        nc.vector.memset(wu[:], 0.0)
        for wi in range(6):
            pwu = kvps.tile([P, 2, 512], f32, tag="pk0", bufs=2)
            mm(pwu[:, 0, :], wu[:, 0:P], wu[:])

        # ======== attention slot machinery ========
        fin_steps = []

        def make_finalize(k, Rt, Lsb, Msb, t2pre=None, chs=(0, 1),
                          st=None, lacc=None, tail=False):
            if st is None:
                st = {}

            def step_rts():
                if len(chs) == 2:
                    nc.sync.dma_start(out=rts[k * P:(k + 1) * P, :],
                                      in_=Rt[:])
                else:
                    ch = chs[0]
                    nc.sync.dma_start(
                        out=rts[k * P:(k + 1) * P,
                                ch * 512:(ch + 1) * 512],
                        in_=Rt[:, ch * 512:(ch + 1) * 512])

            def step_gbat():
                if "attb" not in st:
                    st["attb"] = att_pool.tile([P, NCH, 512], f16,
                                               tag="attb", name="attb")
                attb = st["attb"]
                if t2pre is None:
                    # pv banks are idle during the next slot's mem
                    # portion — keep finalize psums out of the pk0
                    # rotation so scores/exp never stall behind them
                    pgbs = {}
                    for ch in chs:
                        pgb = kvps.tile([P, 512], f32, tag=f"pv{ch}",
                                        bufs=1, name="pgb")
                        mm(pgb[:], ones_r16[:],
                           gate1[0:1, k, ch * 512:(ch + 1) * 512])
                        pgbs[ch] = pgb
                    t2s = {}
                    for ch in chs:
                        t2 = cmb.tile([P, 512], f32, tag="t2")
                        nc.vector.tensor_tensor(t2[:], Msb[:, ch, :],
                                                pgbs[ch][:], OP.mult)
                        t2s[ch] = t2
                else:
                    t2s = t2pre
                for ch in chs:
                    lsrc = lacc[ch][:] if lacc else Lsb[:, ch, :]
                    nc.vector.tensor_tensor(attb[:, ch, :],
                                            lsrc, t2s[ch][:],
                                            OP.add)
                if debug and 1 in chs:
                    nc.gpsimd.dma_start(
                        out=dbg["d_att"][:, k * THALF:(k + 1) * THALF],
                        in_=attb[:].rearrange("p a b -> p (a b)"))

            def step_y(ot):
                def go():
                    attb = st["attb"]
                    pys = {}
                    for ch in chs:
                        py = kvps.tile([P, 512], f32, tag=f"pv{ch}",
                                       bufs=1, name="py")
                        mm(py[:],
                           wot[:, k * C + ot * P:k * C + (ot + 1) * P],
                           attb[:, ch, :])
                        pys[ch] = py
                    yt = ysb.tile([P, NCH, 512], f16, tag="y")
                    # alternate output rings so the tail drains 2x faster
                    # (gpsimd, not scalar: ACT is saturated with exps)
                    eng = nc.sync if ot % 2 == 0 else nc.gpsimd
                    for ch in chs:
                        nc.vector.tensor_copy(out=yt[:, ch, :],
                                              in_=pys[ch][:])
                    if len(chs) == 2:
                        eng.dma_start(
                            out=yp[k * C + ot * P:k * C + (ot + 1) * P,
                                   :],
                            in_=yt[:].rearrange("p a b -> p (a b)"))
                    else:
                        ch = chs[0]
                        nc.sync.dma_start(
                            out=yp[k * C + ot * P:k * C + (ot + 1) * P,
                                   ch * 512:(ch + 1) * 512],
                            in_=yt[:, ch, :])
                return go

            return ([step_rts, step_gbat]
                    + [step_y(ot) for ot in range(NCT)])

        class Slot:
            def __init__(self, k):
                self.k = k
                self.kh = kh0 if k < 2 else kh1
                self.voff = 0 if k < 2 else P
                self.loc_end = 8 if k == 0 else NLOC
                self.msk_lo = {0: 0, 1: 8, 2: 0}[k]
                self.Rt = vec.tile([P, THALF], f16, tag="R", name="Rt")
                self.Lsb = att_pool.tile([P, NCH, 512], f32, tag="Lsb",
                                         name="Lsb")
                self.Msb = att_pool.tile([P, NCH, 512], f32, tag="Msb",
                                         name="Msb")
                self.qrhs = qsb[:, k, :]
                self.pacc = {}
                self.Et = {}
                self.pend = []
                self.idx = 0
                self.t2pre = None

            def skip_ch0(self, j):
                # (tile, ch0) combos fully causal-masked on EVERY core:
                # drop their scores/bias/AV matmuls and narrow the exp
                k = self.k
                if SKIP_MASKED == 0:
                    return False
                return (k == 0 and 4 <= j < 8) or (k >= 1 and 12 <= j)

            def last_loc(self, ch):
                # ch0's local AV group ends early (later tiles skipped)
                if ch == 0:
                    return 3 if self.k == 0 else 11
                return self.loc_end - 1

            def emit_av(self, j, E2x=None):
                k, voff = self.k, self.voff
                spn = min(P, S - j * P)
                E2 = E2x if E2x is not None else self.Et.pop(j)
                reg = 'l' if j < NLOC else 'm'
                first = j == 0 or j == NLOC
                sk0 = reg == 'l' and self.skip_ch0(j)
                for ch in range(NCH):
                    if ch == 0 and sk0:
                        continue
                    if first:
                        tag = ("pg" if ch == 0 else "av1") if reg == 'm' \
                            else f"pv{ch}"
                        self.pacc[(ch, reg)] = kvps.tile(
                            [P, 512], f32, tag=tag, bufs=1,
                            name=f"p{reg}{ch}")
                    last = j == NT - 1 if reg == 'm' else \
                        j == self.last_loc(ch)
                    mm(self.pacc[(ch, reg)][:], vh[:spn, j, voff:voff + P],
                       E2[:spn, ch * 512:(ch + 1) * 512],
                       start=first, stop=last)
                if reg == 'm' and j == NT - 1:
                    # free the mem accumulator banks for the local block
                    for ch in range(NCH):
                        nc.vector.tensor_copy(
                            out=self.Msb[:, ch, :],
                            in_=self.pacc.pop((ch, 'm'))[:])

            def boundary_flush(self):
                # m->l boundary: flush mem AVs (and the Msb copy) ahead
                # of the local Rt adds on the DVE queue
                while self.pend and self.pend[0] >= NLOC:
                    self.emit_av(self.pend.pop(0))
                if self.k == 2:
                    # tail slot: gate*M ready now; shortens the
                    # post-loop critical chain.  The m-acc banks
                    # (pg/av1) were just freed by the Msb copies — use
                    # them so the local tiles' scores rotation (pk0)
                    # is not disturbed.
                    self.t2pre = []
                    for ch, tg in ((0, "pg"), (1, "av1")):
                        pgb = kvps.tile([P, 512], f32, tag=tg, bufs=1,
                                        name=f"pgb2{ch}")
                        mm(pgb[:], ones_r16[:],
                           gate1[0:1, self.k, ch * 512:(ch + 1) * 512])
                        t2 = cmb.tile([P, 512], f32, tag="t2")
                        nc.vector.tensor_tensor(
                            t2[:], self.Msb[:, ch, :], pgb[:],
                            OP.mult)
                        self.t2pre.append(t2)

            def tile_(self, j, stash_idx=None):
                k = self.k
                spn = min(P, S - j * P)
                masked = self.msk_lo <= j < self.loc_end
                skip1 = masked and \
                    (j - self.msk_lo if k < 2 else j) <= 3
                sk0 = j < NLOC and self.skip_ch0(j)
                ps = kvps.tile([P, NCH, 512], f32, tag="pk0", bufs=2,
                               name="ps")
                for ch in range(NCH):
                    if ch == 0 and sk0:
                        continue
                    mm(ps[:spn, ch, :], self.kh[:, j * P:j * P + spn],
                       self.qrhs[:, ch * 512:(ch + 1) * 512],
                       start=True,
                       stop=(not masked) or (ch == 1 and skip1))
                    if masked and not (ch == 1 and skip1):
                        gtab = g2 if k == 2 else g01
                        base = ((THALF if k == 0 else 2 * THALF)
                                - 128 * j + ch * 512)
                        mm(ps[:spn, ch, :], ident[:, :],
                           gtab[:, base:base + 512],
                           start=False, stop=True)
                if stash_idx is not None:
                    # deferred-AV tile: exp lands in the stash; AV later
                    E2 = estash[:, stash_idx, :]
                    nc.scalar.activation(E2[:spn], ps[:spn].rearrange(
                        "p a b -> p (a b)"), AF.Exp, scale=DSCALE)
                    if self.idx == 0:
                        nc.vector.tensor_copy(out=self.Rt[:, :],
                                              in_=E2[:, :])
                    else:
                        nc.vector.tensor_tensor(self.Rt[:spn, :],
                                                self.Rt[:spn, :],
                                                E2[:spn, :], OP.add)
                    self.idx += 1
                    return
                E2 = ep.tile([P, THALF], f16, tag="E")
                if sk0:
                    nc.scalar.activation(E2[:spn, 512:], ps[:spn, 1, :],
                                         AF.Exp, scale=DSCALE)
                    nc.vector.tensor_tensor(self.Rt[:spn, 512:],
                                            self.Rt[:spn, 512:],
                                            E2[:spn, 512:], OP.add)
                else:
                    nc.scalar.activation(E2[:spn], ps[:spn].rearrange(
                        "p a b -> p (a b)"), AF.Exp, scale=DSCALE)
                    if self.idx == 0:
                        nc.vector.tensor_copy(out=self.Rt[:, :],
                                              in_=E2[:, :])
                    else:
                        nc.vector.tensor_tensor(self.Rt[:spn, :],
                                                self.Rt[:spn, :],
                                                E2[:spn, :], OP.add)
                self.Et[j] = E2
                self.pend.append(j)
                self.idx += 1
                if len(self.pend) > 6:
                    self.emit_av(self.pend.pop(0))

            def finish_avs(self):
                for j in self.pend:
                    self.emit_av(j)
                self.pend = []
                if self.k == 2:
                    # last slot: leave the L accumulators in psum; the
                    # finalize attb add reads them directly (saves two
                    # DVE copies on the final critical chain)
                    self.lacc = {ch: self.pacc.pop((ch, 'l'))
                                 for ch in range(NCH)
                                 if (ch, 'l') in self.pacc}
                    return
                for ch in range(NCH):
                    if (ch, 'l') in self.pacc:
                        nc.vector.tensor_copy(
                            out=self.Lsb[:, ch, :],
                            in_=self.pacc.pop((ch, 'l'))[:])

        # ======== phase 1: KV chunks (local first), slot-0 interleave ==
        slot0 = Slot(0)
        slot1 = Slot(1)
        lci = 0
        next_subs = [kvt0a, kvt0b]
        for ci, (off, w) in enumerate(CHUNKS):
            is_loc = off < T
            kv_subs = next_subs
            # prefetch chunk ci+1 now; guard DMAs (tiny SBUF->SBUF reads
            # of the current chunk) keep each ring serialized so the
            # in-flight set stays small and early transfers finish early
            if ci + 1 < len(CHUNKS):
                nc.sync.dma_start(out=gscr[:], in_=kv_subs[0][0:1, 0, 0:8])
                next_subs = []
                for subi, (sb, so, sw) in enumerate(CHUNK_SUBS[ci + 1]):
                    eng = nc.sync if subi == 0 else nc.scalar
                    kvst = kvp.tile([P, NCT, 512], f16,
                                    tag=f"kv{subi}", name=f"kv{subi}")
                    eng.dma_start(
                        out=kvst[:, :, :sw],
                        in_=kvsrc[:, sb:sb + NCT * sw].rearrange(
                            "p (a s) -> p a s", a=NCT))
                    next_subs.append(kvst)
            if ci == 1:
                # non-critical consts, gated behind chunk-1 data
                nc.gpsimd.tensor_copy(out=lateg[:],
                                      in_=kv_subs[0][0:1, 0, 0:8])
                nc.gpsimd.dma_start(
                    out=xq2[:],
                    in_=dram["xq2"][:, :].rearrange(
                        "p (a s) -> p a s", a=NCT))
                nc.gpsimd.dma_start(out=wot[:], in_=dram["wot"][:, :])
                nc.gpsimd.dma_start(out=ident[:], in_=dram["ident"][:, :])
                nc.gpsimd.dma_start(out=g01[:], in_=dram["g01"][:, :])
                nc.gpsimd.dma_start(out=g2[:], in_=dram["g2"][:, :])
            # pass 1: K for all subs (needs only wk0/wk1 + kv data).
            # For chunk 0, interleave ct-halves across subs: cts 0-2 land
            # on the sync ring, 3-5 on scalar — emit all ct0-2 matmuls
            # (both subs) before any ct3-5 so the PE FIFO never stalls
            # on the slower-arriving half.
            pks = []
            for subi, (sb, so, sw) in enumerate(CHUNK_SUBS[ci]):
                pks.append(kvps.tile([P, 2, 512], f32, tag="pk0", bufs=2,
                                     name="pk"))
            ct_order = ([(cth, subi) for cth in (0, 1)
                         for subi in range(len(CHUNK_SUBS[ci]))]
                        if ci == 0 else
                        [(cth, subi)
                         for subi in range(len(CHUNK_SUBS[ci]))
                         for cth in (0, 1)])
            for oi, (cth, subi) in enumerate(ct_order):
                sb, so, sw = CHUNK_SUBS[ci][subi]
                kv_t = kv_subs[subi]
                pk = pks[subi]
                for ct in range(3 * cth, 3 * cth + 3):
                    kvs = kv_t[:, ct, :sw]
                    mm(pk[:, 0, :sw], wk0[:, ct, :], kvs,
                       start=(ct == 0), stop=(ct == NCT - 1))
                    mm(pk[:, 1, :sw], wk1[:, ct, :], kvs,
                       start=(ct == 0), stop=(ct == NCT - 1))
                if cth == 1:
                    nc.scalar.copy(kh0[:, off + so:off + so + sw],
                                   pk[:, 0, :sw])
                    nc.scalar.copy(kh1[:, off + so:off + so + sw],
                                   pk[:, 1, :sw])
            # pass 2: V for all subs
            for subi, (sb, so, sw) in enumerate(CHUNK_SUBS[ci]):
                kv_t = kv_subs[subi]
                nsub = []
                o3 = 0
                while o3 < sw:
                    nsub.append((o3, min(P, sw - o3)))
                    o3 += P
                pv = [kvps.tile([P, 2 * 2 * P], f32, tag=f"pv{vi}",
                                name=f"pv{vi}", bufs=1)
                      for vi in range((len(nsub) + 1) // 2)]
                for ct in range(NCT):
                    for si, (o3, sn) in enumerate(nsub):
                        co = (si % 2) * 2 * P
                        nc.tensor.matmul(
                            pv[si // 2][:sn, co:co + 2 * P],
                            kv_t[:, ct, o3:o3 + sn],
                            wv2[:, ct, :],
                            start=(ct == 0 and si % 2 == 0),
                            stop=(ct == NCT - 1),
                            skip_group_check=True)
                for si, (o3, sn) in enumerate(nsub):
                    jj = (off + so + o3) // P
                    co = (si % 2) * 2 * P
                    nc.vector.tensor_copy(
                        out=vh[:sn, jj, :],
                        in_=pv[si // 2][:sn, co:co + 2 * P])
            # pass 3: q + gate projections (local chunks only)
            if is_loc:
                for subi, (sb, so, sw) in enumerate(CHUNK_SUBS[ci]):
                    kv_t = kv_subs[subi]
                    pq = kvps.tile([P, 2, 512], f32, tag="pk0", bufs=2,
                                   name="pq")
                    pg = kvps.tile([P, 512], f32, tag="pg", bufs=1,
                                   name="pg")
                    cki = lci * 2 + so // 512   # local 512-chunk 0..3
                    for ct in range(NCT):
                        kvs = kv_t[:, ct, :sw]
                        mm(pq[:, 0, :], wq3[:, ct, 0:P], kvs,
                           start=(ct == 0), stop=(ct == NCT - 1))
                        mm(pg[:3, :], wfg[:, ct, cki, :], kvs,
                           start=(ct == 0), stop=(ct == NCT - 1))
                    own = 0 if cki < 2 else 1
                    colh = (cki % 2) * 512
                    nc.vector.tensor_scalar_add(
                        qsb[:, own, colh:colh + 512], pq[:, 0, :],
                        qbs[:, own:own + 1])
                    if cki < 2:
                        nc.vector.tensor_copy(
                            out=gacc[:, colh:colh + 512], in_=pg[:3, :])
                    else:
                        nc.vector.tensor_tensor(
                            gacc[:, colh:colh + 512],
                            gacc[:, colh:colh + 512], pg[:3, :], OP.add)
            if is_loc:
                lci += 1
                if lci == N_LOC_CHUNKS:
                    nc.scalar.activation(gate[:], gacc[:], AF.Sigmoid,
                                         bias=gb3[:, 0:1], scale=1.0)
                    nc.sync.dma_start(out=gate1[:], in_=gate[:])
            else:
                # interleave slot-0 mem attention tiles for this chunk,
                # plus slot-1 deferred-AV tiles (exp-only) for the tail
                for j in range(off // P, (off + w + P - 1) // P):
                    slot0.tile_(j)
                    if j in STASH_JS:
                        slot1.tile_(j, stash_idx=j - STASH_JS[0])

        if debug:
            nc.gpsimd.dma_start(out=dbg["d_q"][:, :],
                                in_=qsb[:].rearrange("p a b -> p (a b)"))
            nc.sync.dma_start(out=dbg["d_gate"][:, :], in_=gate[:])
            nc.gpsimd.dma_start(out=dbg["d_kh0"][:, :], in_=kh0[:, 0:1024])
            nc.gpsimd.dma_start(out=dbg["d_vh"][:, :],
                                in_=vh[:, 0:2, :].rearrange(
                                    "p a b -> p (a b)"))

        # ======== phase 2: slot-0 local tiles, then slots 1, 2 ========
        slot0.boundary_flush()
        for j in range(slot0.loc_end):
            slot0.tile_(j)
        slot0.finish_avs()
        fin_steps = make_finalize(0, slot0.Rt, slot0.Lsb, slot0.Msb)

        for k in (1, 2):
            sl = slot1 if k == 1 else Slot(k)
            js = [j for j in range(NLOC, NT)
                  if not (k == 1 and j in STASH_JS)] \
                + list(range(sl.loc_end))
            s0steps = []
            st2 = {}
            for idx, j in enumerate(js):
                if fin_steps and idx >= 2 and idx % 3 == 2:
                    fin_steps.pop(0)()
                if k == 1 and idx in (2, 4):
                    # slot-2 q projection in slot-1's PE slack (pv bank:
                    # out of the scores psum rotation)
                    so2 = (idx - 2) * 256
                    pq2 = kvps.tile([P, 512], f32, tag="pv0", bufs=1,
                                    name="pq2")
                    for ct in range(NCT):
                        mm(pq2[:], wq3[:, ct, P:2 * P],
                           xq2[:, ct, so2:so2 + 512],
                           start=(ct == 0), stop=(ct == NCT - 1))
                    nc.vector.tensor_scalar_add(
                        qsb[:, 2, so2:so2 + 512], pq2[:],
                        qbs[:, 2:3])
                if j < NLOC and (idx == 0 or js[idx - 1] >= NLOC):
                    sl.boundary_flush()
                    if k == 1:
                        # drain the deferred-AV stash (tile NT-1 last:
                        # it closes the mem accumulator group)
                        for sj in STASH_JS:
                            sl.emit_av(sj,
                                       E2x=estash[:, sj - STASH_JS[0], :])
                sl.tile_(j)
                if k == 2 and j == 11 and EARLY_FIN and SKIP_MASKED:
                    # ch0 (first 512 token cols) is final after tile 11:
                    # finalize it while tiles 12-15 (ch1-only) proceed
                    while sl.pend and sl.pend[0] <= 11:
                        sl.emit_av(sl.pend.pop(0))
                    nc.vector.tensor_copy(
                        out=sl.Lsb[:, 0, :],
                        in_=sl.pacc.pop((0, 'l'))[:])
                    s0steps = make_finalize(2, sl.Rt, sl.Lsb, sl.Msb,
                                            sl.t2pre, chs=(0,), st=st2)
                elif k == 2 and j > 11 and s0steps:
                    s0steps.pop(0)()
                    if s0steps:
                        s0steps.pop(0)()
            sl.finish_avs()
            for st_ in s0steps:
                st_()
            for st_ in fin_steps:   # drain any leftover steps
                st_()
            if k == 2 and EARLY_FIN and SKIP_MASKED:
                fin_steps = make_finalize(k, sl.Rt, sl.Lsb, sl.Msb,
                                          sl.t2pre, chs=(1,), st=st2)
            elif k == 2:
                fin_steps = make_finalize(k, sl.Rt, sl.Lsb, sl.Msb,
                                          sl.t2pre, lacc=sl.lacc,
                                          tail=True)
            else:
                fin_steps = make_finalize(k, sl.Rt, sl.Lsb, sl.Msb)
        for st_ in fin_steps:
            st_()
    nc.compile()
    return nc


def make_in_maps(x, forward_memory, reverse_memory, ctrl, Wq, Wk, Wv, Wo,
                 Wc, Wg, bg):
    f = np.float32
    h = np.float16

    def sb6(a):
        """[C, m] -> [128, 6*m] feature-tile-major SBUF layout."""
        m = a.shape[1]
        return np.ascontiguousarray(
            a.reshape(NCT, P, m).transpose(1, 0, 2).reshape(P, NCT * m))

    def kvt_slabs(kvT_cs):
        """[C, S] -> [128, KVT_COLS] chunk-major contiguous slabs."""
        tiled = kvT_cs.reshape(NCT, P, S)          # [ct, p, s]
        blocks = []
        for ci in range(len(CHUNKS)):
            off = CHUNKS[ci][0]
            for sb, so, sw in CHUNK_SUBS[ci]:
                blk = tiled[:, :, off + so:off + so + sw]   # [ct, p, sw]
                blocks.append(blk.transpose(1, 0, 2).reshape(P, NCT * sw))
        return np.ascontiguousarray(np.concatenate(blocks, axis=1))

    BIG = np.float16(-30000.0)
    rr_ = np.arange(P).reshape(P, 1)
    v01 = np.arange(-THALF, THALF).reshape(1, 2 * THALF)
    g01 = np.where(v01 < rr_, BIG, np.float16(0.0)).astype(h)
    v2 = np.arange(-2 * THALF, THALF).reshape(1, 3 * THALF)
    ident = np.eye(P, dtype=h)
    qb_full = (np.asarray(ctrl, f) @ np.asarray(Wc, f).T)  # [C]

    in_maps = []
    for core in range(8):
        b, g = core // 4, core % 4
        hp, hs, hsh = GROUP_MAP[g]
        kv = np.concatenate(
            [x[b], forward_memory[b], reverse_memory[b]], axis=0)
        kvT = np.ascontiguousarray(kv.T, dtype=f)          # [C, S]
        xq2 = np.ascontiguousarray(
            x[b, hsh * THALF:(hsh + 1) * THALF].T, dtype=f)  # [C, THALF]
        # q weights: own (pair head) and single head
        wq_own = np.ascontiguousarray(Wq[hp * P:(hp + 1) * P, :].T, f)
        wq_s2 = np.ascontiguousarray(Wq[hs * P:(hs + 1) * P, :].T, f)
        zA = 1.0 if hsh == 0 else 0.0
        zB = 1.0 if hsh == 1 else 0.0
        wq3 = np.concatenate([wq_own, wq_s2], axis=1)
        # fused gate weights: gate logit for head hh at token t =
        # (Wg[hh]@Wq) . x_t + const
        wf = np.asarray(Wg, f) @ np.asarray(Wq, f)         # [H, C]
        wf_own = wf[hp]                                    # [C]
        wf_s2A = wf[hs] * zA
        wf_s2B = wf[hs] * zB
        z = np.zeros(C, f)
        wfg = np.zeros((C, 4, 3), f)
        for ckk in range(4):
            wfg[:, ckk, 0] = wf_own if ckk < 2 else z
            wfg[:, ckk, 1] = wf_own if ckk >= 2 else z
            wfg[:, ckk, 2] = wf_s2A if ckk < 2 else wf_s2B
        units = slot_units(g)
        wvT2 = np.concatenate(
            [np.ascontiguousarray(Wv[hh * P:(hh + 1) * P, :].T)
             for hh in (hp, hs)], axis=1)
        wot = np.concatenate(
            [np.ascontiguousarray(Wo[:, hh * P:(hh + 1) * P].T)
             for (hh, _) in units], axis=1)
        qbs = np.stack([qb_full[hh * P:(hh + 1) * P]
                        for (hh, _) in units], axis=1).astype(f)
        gb3 = np.zeros((3, 1), f)
        for kslot, (hh, _) in enumerate(units):
            gb3[kslot, 0] = float(np.asarray(Wg, f)[hh] @ qb_full
                                  + np.asarray(bg, f)[hh])
        g2 = np.where(v2 < rr_ - THALF * hsh, BIG,
                      np.float16(0.0)).astype(h)
        in_maps.append({
            "kvT": kvt_slabs(kvT).astype(h),
            "xq2": sb6(xq2).astype(h),
            "wk0": sb6(np.ascontiguousarray(
                Wk[hp * P:(hp + 1) * P, :].T, f)).astype(h),
            "wk1": sb6(np.ascontiguousarray(
                Wk[hs * P:(hs + 1) * P, :].T, f)).astype(h),
            "wv2": sb6(np.ascontiguousarray(wvT2, f)).astype(h),
            "wq3": sb6(np.ascontiguousarray(wq3, f)).astype(h),
            "wfg": sb6(np.ascontiguousarray(
                wfg.reshape(C, 12), f)).astype(h),
            "wot": np.ascontiguousarray(wot, f).astype(h),
            "ident": ident, "g01": g01, "g2": g2,
            "qbs": qbs, "gb3": gb3,
        })
    return in_maps


def unshard(results):
    y = np.zeros((B, T, C), dtype=np.float32)
    for core in range(8):
        b, g = core // 4, core % 4
        ypc = results[core]["yp"].astype(np.float32)
        rts = results[core]["rts"].astype(np.float32)
        for kslot, (_, half) in enumerate(slot_units(g)):
            den = rts[kslot * P:(kslot + 1) * P, :].sum(axis=0)  # [THALF]
            y[b, half * THALF:(half + 1) * THALF, :] += \
                (ypc[kslot * C:(kslot + 1) * C, :] / den[None, :]).T
    return y


_nc_cache = {}


def _get_nc(debug=False):
    key = (debug,)
    if key not in _nc_cache:
        _nc_cache[key] = build_nc(debug)
    return _nc_cache[key]


def kernel(**inputs):
    return kernel_ex(**inputs)[0]


def kernel_ex(trace=False, trace_cores=None, debug=False, **inputs):
    from concourse.bass_utils import run_bass_kernel_spmd

    inputs.pop("use_f32r", None)
    inputs.pop("att_bf16", None)
    np_inputs = {k: np.asarray(v) for k, v in inputs.items()}
    in_maps = make_in_maps(**np_inputs)
    nc = _get_nc(debug)
    res = run_bass_kernel_spmd(nc, in_maps, list(range(8)), trace=trace,
                               trace_cores=trace_cores)
    return unshard(res.results), res
